# revision 37
# baseline (speedup 1.0000x reference)
"""Self-attention (IntraSelfAttention) kernel for Trainium2, 8-core data parallel.

Key optimizations over the naive full-[S,S] version:

1. Mask compaction (exact): masked-out rows/columns contribute nothing to
   the output, so each core gathers its batch's unmasked rows (~512 of
   1024) and pads to SC (multiple of 128).  Both matmuls shrink ~4x.
2. fp8 (e4m3) DoubleRow matmuls (2 contraction rows/cycle) for BOTH
   phases.  Accuracy is preserved by:
     - inputs scaled by 8 (avoids fp8 subnormals); exp() applies the
       compensating 1/64 via the activation engine's input scale.
     - AV multiplies X = 16*(exp(QK) - 1) (fp8; small values => small
       absolute error) against V8 = 8*A.  The missing ones*V term
       (E = 1 + X/16) is a per-batch column sum added back on the host
       from TRUE fp32 values, which cancels the bulk of V's quantization.
     - the diagonal term X_ss*V_s (X_ss ~ 100 >> offdiag, so its fp8
       step is coarse) is replaced on the host with the exact
       expm1(|a_s|^2)*a_s.
   Row sums come from an extra (8.0) column in V; normalization happens
   on the host in fp32.
3. fp16 output DMA; partition-major DRAM layouts so every DMA descriptor
   moves multi-KB contiguous chunks per partition; input DMAs on two
   different hardware queues (SP + Activation).
4. QK iterates column-slab-major so the AV phase (which consumes X
   column blocks) can start before the last QK exp() lands; exp/X widths
   are trimmed to EW (real rows padded to 32) since padded output rows
   are discarded on the host anyway.
5. Dependency-free dummy matmuls before the real work ramp the tensor
   engine out of its low p-state (0.65 -> 2.4 GHz takes ~3us of
   continuous execution) while the input DMAs stream.

Hardware constraint shaping the code: most engine instruction structs accept
only ONE sync-wait, so the dep graph is arranged such that no instruction
ever needs two new semaphore waits (et tiles are written once each — no
write-after-read hazards on the Exp; hand-double-buffered PSUM tiles in the
AV phase so slot reuse is same-tile WAW, which is same-engine ordered).
"""

import numpy as np

try:
    import concourse.bass as bass
except ImportError:
    import sys

    sys.path.insert(0, "/opt/trn_rl_repo")
    import concourse.bass as bass

import ml_dtypes
import concourse.mybir as mybir
import concourse.tile as tile
from concourse import bass_utils
from concourse.tile_sem_assignment import PROC_NAME_TO_IDX

_IDX2PROC = {v: k for k, v in PROC_NAME_TO_IDX.items()}


def _split_drain_and_barrier(self, tick_clock, wait_clock):
    """Replacement for TileContext._drain_and_barrier.

    The stock version attaches every outstanding semaphore wait to the single
    kernel-tail Drain instruction; walrus's per-instruction sync-wait capacity
    is tiny, so with >4-ish sems the NEFF fails codegen ("Too many sync wait
    commands"). Split the waits across single-wait sequencer nops instead.
    """
    nc = self.nc

    # Walrus accepts only ONE sync-wait on most engine instruction structs.
    # The tile scheduler occasionally emits a redundant same-engine wait
    # (e.g. a DVE copy waiting on both PE_sem and its own DVE_sem when the
    # producing matmul already waited on that DVE_sem value — a read-after-
    # read ordering with no hazard). Strip self-engine waits when an
    # instruction carries more than one wait.
    for fn in nc.m.functions:
        for blk in fn.blocks:
            for inst in blk.instructions:
                si = getattr(inst, "sync_info", None)
                if si is None or not si.on_wait or len(si.on_wait) < 2:
                    continue
                eng = getattr(inst, "engine", None)
                ename = getattr(eng, "name", str(eng))
                keep = [w for w in si.on_wait if not str(w.ant_name).startswith(f"{ename}_")]
                if keep and len(keep) < len(si.on_wait):
                    si.on_wait = keep

    gc = tick_clock.global_clock
    ticks = list(gc)
    for idx, sem in self.sems.allocated().items():
        tick = ticks[idx]
        if tick <= 0:
            continue
        name = _IDX2PROC.get(idx, "")
        val = tick * (16 if name.startswith("DMA") else 1)
        nc.sync.nop().wait_op(sem, val, "sem-ge")
    nc.sync.drain()
    nc.all_engine_barrier()
    popped = nc._tile_sem_poison_stack.pop()
    assert popped is self._sem_poison
    nc.clear_and_free_semaphores(list(self.sems.allocated().values()))


tile.TileContext._drain_and_barrier = _split_drain_and_barrier

B, S, D = 8, 1024, 768
NCORES = 8
EPS = 1e-7
P = 128
KT = D // P  # 6 k-tiles over D
DV = D + 1  # 768 cols of V | rowsum column
AVW = 776  # device-side V tile width (even/8B-aligned padding of DV)
ASC = 8.0  # input scale (QK inputs and V are stored as 8*A in fp8)
XSC = 16.0  # X = exp(QK)-1 is stored as 16*X in fp8

F8 = mybir.dt.float8e4
F16 = mybir.dt.float16
F32 = mybir.dt.float32
NP8 = ml_dtypes.float8_e4m3
DR = mybir.MatmulPerfMode.DoubleRow
EXPF = mybir.ActivationFunctionType.Exp

_cache = {}


def _build(SC, EW):
    NT = SC // P  # t/row blocks of the compacted sequence
    KTX = 2 * ((NT + 1) // 2)  # AV contraction k-slots, padded to even
    nc = bass.Bass()
    at8 = nc.declare_dram_parameter("at8", [P, KT, SC], F8, isOutput=False)
    av8 = nc.declare_dram_parameter("av8", [P, KTX, AVW], F8, isOutput=False)
    out16 = nc.declare_dram_parameter("out16", [P, NT, DV], F16, isOutput=True)

    # exp/X only need columns for real (unmasked) output rows; the rest of
    # each X row block is zeroed once and the garbage output rows discarded.
    slabs = [(lo, min(512, EW - lo)) for lo in range(0, EW, 512)]

    with tile.TileContext(nc) as tc:
        with (
            tc.tile_pool(name="w", bufs=1) as wpool,
            tc.tile_pool(name="x", bufs=1) as xpool,
            tc.tile_pool(name="e", bufs=2 * NT) as epool,
            tc.tile_pool(name="qkps", bufs=4, space="PSUM") as qkps,
            tc.tile_pool(name="avps", bufs=1, space="PSUM") as avps,
            tc.tile_pool(name="o", bufs=1) as opool,
        ):
            # warm-up operands for the PE p-state ramp (zeroed)
            warm_l = wpool.tile([P, 2, 64], F8, name="warm_l", tag="warm_l")
            nc.gpsimd.memset(warm_l[:, :, :], 0.0)
            warm_r = wpool.tile([P, 2, 512], F8, name="warm_r", tag="warm_r")
            nc.gpsimd.memset(warm_r[:, :, :], 0.0)

            # at8 split across two hardware DMA queues (SP + Activation)
            at8t = wpool.tile([P, KT, SC], F8, name="at8t", tag="at8t")
            KC = 4  # k-tiles in the first chunk (covers DoubleRow pairs 0,1)
            nc.sync.dma_start(at8t[:, 0:KC, :], at8[:, 0:KC, :])
            nc.scalar.dma_start(at8t[:, KC:KT, :], at8[:, KC:KT, :])
            av8t = wpool.tile([P, KTX, AVW], F8, name="av8t", tag="av8t")
            nc.scalar.dma_start(av8t[:, :, :], av8[:, :, :])

            # X tiles; k-slot NT..KTX-1 stays zero (contraction padding)
            xall = xpool.tile([P, KTX, SC], F8, name="xall", tag="xall")
            for t in range(NT, KTX):
                nc.gpsimd.memset(xall[:, t, :], 0.0)
            if EW < SC:
                for t in range(NT):
                    nc.gpsimd.memset(xall[:, t, EW:SC], 0.0)

            # --- PE warm-up: dependency-free dummy matmuls keep the tensor
            #     engine busy (ramping its p-state) while the inputs stream ---
            wps = qkps.tile([P, 512], F32, tag="qk", name="qk_warm")
            for _ in range(6):
                nc.tensor.matmul(
                    wps[0:64, :], warm_l[:, :, :], warm_r[:, :, :], perf_mode=DR
                )

            # --- QK phase (slab-major): psum = (8A)(8A)^T block,
            #     X = 16*(exp(psum/64) - 1).
            # The pool-engine -1/scale pass runs in 128-column chunks,
            # column-major, so each AV m-block's weight load waits only on
            # the two chunk conversions it actually reads — the AV phase
            # starts as soon as the last row block's first chunk lands. ---
            ets = {}
            for lo, w in slabs:
                for i in range(NT):
                    ps = qkps.tile([P, 512], F32, tag="qk", name=f"qk_{i}_{lo}")
                    for q in range(KT // 2):
                        nc.tensor.matmul(
                            ps[:, :w],
                            at8t[:, 2 * q : 2 * q + 2, i * P : (i + 1) * P],
                            at8t[:, 2 * q : 2 * q + 2, lo : lo + w],
                            start=(q == 0),
                            stop=(q == KT // 2 - 1),
                            perf_mode=DR,
                        )
                    et = epool.tile([P, 512], F16, tag="et", name=f"et_{i}_{lo}")
                    nc.scalar.activation(
                        et[:, :w], ps[:, :w], EXPF, scale=1.0 / (ASC * ASC)
                    )
                    ets[(lo, i)] = et
                    nc.gpsimd.tensor_scalar(
                        xall[:, i, lo : lo + w],
                        et[:, :w],
                        -1.0,
                        XSC,
                        mybir.AluOpType.add,
                        mybir.AluOpType.mult,
                    )

            # --- AV phase: P_ext[m] = X^T-block @ [8V | 8*ones] ---
            ots = [
                opool.tile([P, DV], F16, name=f"og{m}", tag=f"og{m}")
                for m in range(NT)
            ]
            pabuf = [
                avps.tile([P, 512], F32, tag=f"pa{x}", name=f"pa{x}") for x in range(2)
            ]
            pbbuf = [
                avps.tile([P, 258], F32, tag=f"pb{x}", name=f"pb{x}") for x in range(2)
            ]
            NQ = KTX // 2  # DoubleRow contraction pairs (incl. zero padding)
            for m in range(NT):
                pa = pabuf[m % 2]
                pb = pbbuf[m % 2]
                for q in range(NQ):
                    lt = xall[:, 2 * q : 2 * q + 2, m * P : (m + 1) * P]
                    nc.tensor.matmul(
                        pa[:, :],
                        lt,
                        av8t[:, 2 * q : 2 * q + 2, 0:512],
                        start=(q == 0),
                        stop=(q == NQ - 1),
                        perf_mode=DR,
                    )
                    nc.tensor.matmul(
                        pb[:, :],
                        lt,
                        av8t[:, 2 * q : 2 * q + 2, 512:770],
                        start=(q == 0),
                        stop=(q == NQ - 1),
                        perf_mode=DR,
                    )
                nc.vector.tensor_scalar_add(ots[m][:, 0:512], pa[:, :], 0.0)
                nc.vector.tensor_scalar_add(ots[m][:, 512:DV], pb[:, 0:257], 0.0)
                # ship only partitions holding real (unmasked) rows; the tail
                # block is mostly padding (the DRAM output is zero-initialized
                # and the host discards pad rows anyway)
                lp = min(P, EW - m * P)
                nc.sync.dma_start(out16[0:lp, m, :], ots[m][0:lp, :])

    return nc


def _get_nc(SC, EW):
    if (SC, EW) not in _cache:
        _cache[(SC, EW)] = _build(SC, EW)
    return _cache[(SC, EW)]


def kernel(input_a, input_mask, _trace=False, **_kw):
    A = np.asarray(input_a, dtype=np.float32)  # [B, S, D]
    M = np.asarray(input_mask)  # [B, S] int32

    nb, s, d = A.shape
    idxs = [np.nonzero(M[b] != 0)[0] for b in range(nb)]
    maxn = max(1, max(len(ix) for ix in idxs))
    SC = max(P, -(-maxn // P) * P)  # pad count to multiple of 128
    EW = min(SC, -(-maxn // 32) * 32)  # exp width: real rows padded to 32
    NT = SC // P
    KT_ = d // P
    KTX = 2 * ((NT + 1) // 2)

    in_maps = []
    hosts = []
    for b in range(nb):
        ix = idxs[b]
        n = len(ix)
        Ac = np.zeros((SC, d), np.float32)
        Ac[:n] = A[b][ix]
        a8 = (ASC * Ac).astype(NP8)  # [SC, d] fp8 of 8*A
        a8f = a8.astype(np.float32)
        at8 = np.ascontiguousarray(
            a8.T.reshape(KT_, P, SC).transpose(1, 0, 2)
        )
        av8 = np.zeros((P, KTX, AVW), NP8)
        blk = np.zeros((SC, AVW), NP8)
        blk[:, :d] = a8
        blk[:n, d] = ASC
        av8[:, 0:NT, :] = blk.reshape(NT, P, AVW).transpose(1, 0, 2)
        in_maps.append({"at8": at8, "av8": av8})

        # host-side correction data
        colsum = Ac[:n].sum(axis=0)  # true fp32 column sums
        qk_ss = np.einsum("ij,ij->i", a8f[:n], a8f[:n])
        x_dev = (
            (
                (
                    np.exp(qk_ss / (ASC * ASC)).astype(np.float16).astype(np.float32)
                    - 1.0
                )
                * XSC
            ).astype(NP8)
        ).astype(np.float32) / XSC
        a64 = Ac[:n].astype(np.float64)
        x_ideal = np.expm1(np.einsum("ij,ij->i", a64, a64)).astype(np.float32)
        hosts.append((colsum, x_dev, x_ideal, a8f))

    nc = _get_nc(SC, EW)
    res = bass_utils.run_bass_kernel_spmd(
        nc, in_maps, core_ids=list(range(NCORES)), trace=_trace
    )

    scale = 1.0 / (ASC * XSC)
    out = np.zeros((nb, s, d), np.float32)
    for b in range(nb):
        ix = idxs[b]
        n = len(ix)
        colsum, x_dev, x_ideal, a8f = hosts[b]
        R = (
            res.results[b]["out16"]
            .transpose(1, 0, 2)
            .reshape(SC, DV)
            .astype(np.float32)
        )
        Vq = a8f[:n] / ASC
        Ac_n = A[b][ix]
        U = (
            R[:n, :d] * scale
            + colsum[None, :]
            - x_dev[:, None] * Vq
            + x_ideal[:, None] * Ac_n
        )
        rs = R[:n, d] * scale + n + (x_ideal - x_dev)
        out[b][ix] = U / (rs + EPS)[:, None]
    if _trace:
        kernel.last_results = res
    return out


# revision 38
# speedup vs baseline: 1.0188x; 1.0188x over previous
"""Self-attention (IntraSelfAttention) kernel for Trainium2, 8-core data parallel.

Key optimizations over the naive full-[S,S] version:

1. Mask compaction (exact): masked-out rows/columns contribute nothing to
   the output, so each core gathers its batch's unmasked rows (~512 of
   1024) and pads to SC (multiple of 128).  Both matmuls shrink ~4x.
2. fp8 (e4m3) DoubleRow matmuls (2 contraction rows/cycle) for BOTH
   phases.  Accuracy is preserved by:
     - inputs scaled by 8 (avoids fp8 subnormals); exp() applies the
       compensating 1/64 via the activation engine's input scale.
     - AV multiplies X = 16*(exp(QK) - 1) (fp8; small values => small
       absolute error) against V8 = 8*A.  The missing ones*V term
       (E = 1 + X/16) is a per-batch column sum added back on the host
       from TRUE fp32 values, which cancels the bulk of V's quantization.
     - the diagonal term X_ss*V_s (X_ss ~ 100 >> offdiag, so its fp8
       step is coarse) is replaced on the host with the exact
       expm1(|a_s|^2)*a_s.
   Row sums come from an extra (8.0) column in V; normalization happens
   on the host in fp32.
3. fp16 output DMA; partition-major DRAM layouts so every DMA descriptor
   moves multi-KB contiguous chunks per partition; input DMAs on two
   different hardware queues (SP + Activation).
4. QK iterates column-slab-major so the AV phase (which consumes X
   column blocks) can start before the last QK exp() lands; exp/X widths
   are trimmed to EW (real rows padded to 32) since padded output rows
   are discarded on the host anyway.
5. Dependency-free dummy matmuls before the real work ramp the tensor
   engine out of its low p-state (0.65 -> 2.4 GHz takes ~3us of
   continuous execution) while the input DMAs stream.

Hardware constraint shaping the code: most engine instruction structs accept
only ONE sync-wait, so the dep graph is arranged such that no instruction
ever needs two new semaphore waits (et tiles are written once each — no
write-after-read hazards on the Exp; hand-double-buffered PSUM tiles in the
AV phase so slot reuse is same-tile WAW, which is same-engine ordered).
"""

import numpy as np

try:
    import concourse.bass as bass
except ImportError:
    import sys

    sys.path.insert(0, "/opt/trn_rl_repo")
    import concourse.bass as bass

import ml_dtypes
import concourse.mybir as mybir
import concourse.tile as tile
from concourse import bass_utils
from concourse.tile_sem_assignment import PROC_NAME_TO_IDX

_IDX2PROC = {v: k for k, v in PROC_NAME_TO_IDX.items()}


def _split_drain_and_barrier(self, tick_clock, wait_clock):
    """Replacement for TileContext._drain_and_barrier.

    The stock version attaches every outstanding semaphore wait to the single
    kernel-tail Drain instruction; walrus's per-instruction sync-wait capacity
    is tiny, so with >4-ish sems the NEFF fails codegen ("Too many sync wait
    commands"). Split the waits across single-wait sequencer nops instead.
    """
    nc = self.nc

    # Walrus accepts only ONE sync-wait on most engine instruction structs.
    # The tile scheduler occasionally emits a redundant same-engine wait
    # (e.g. a DVE copy waiting on both PE_sem and its own DVE_sem when the
    # producing matmul already waited on that DVE_sem value — a read-after-
    # read ordering with no hazard). Strip self-engine waits when an
    # instruction carries more than one wait.
    for fn in nc.m.functions:
        for blk in fn.blocks:
            for inst in blk.instructions:
                si = getattr(inst, "sync_info", None)
                if si is None or not si.on_wait or len(si.on_wait) < 2:
                    continue
                eng = getattr(inst, "engine", None)
                ename = getattr(eng, "name", str(eng))
                keep = [w for w in si.on_wait if not str(w.ant_name).startswith(f"{ename}_")]
                if keep and len(keep) < len(si.on_wait):
                    si.on_wait = keep

    gc = tick_clock.global_clock
    ticks = list(gc)
    for idx, sem in self.sems.allocated().items():
        tick = ticks[idx]
        if tick <= 0:
            continue
        name = _IDX2PROC.get(idx, "")
        val = tick * (16 if name.startswith("DMA") else 1)
        nc.sync.nop().wait_op(sem, val, "sem-ge")
    nc.sync.drain()
    nc.all_engine_barrier()
    popped = nc._tile_sem_poison_stack.pop()
    assert popped is self._sem_poison
    nc.clear_and_free_semaphores(list(self.sems.allocated().values()))


tile.TileContext._drain_and_barrier = _split_drain_and_barrier

B, S, D = 8, 1024, 768
NCORES = 8
EPS = 1e-7
P = 128
KT = D // P  # 6 k-tiles over D
DV = D + 1  # 768 cols of V | rowsum column
AVW = 776  # device-side V tile width (even/8B-aligned padding of DV)
ASC = 8.0  # input scale (QK inputs and V are stored as 8*A in fp8)
XSC = 16.0  # X = exp(QK)-1 is stored as 16*X in fp8

F8 = mybir.dt.float8e4
F16 = mybir.dt.float16
F32 = mybir.dt.float32
NP8 = ml_dtypes.float8_e4m3
DR = mybir.MatmulPerfMode.DoubleRow
EXPF = mybir.ActivationFunctionType.Exp

_cache = {}


def _build(SC, EW):
    NT = SC // P  # t/row blocks of the compacted sequence
    KTX = 2 * ((NT + 1) // 2)  # AV contraction k-slots, padded to even
    nc = bass.Bass()
    at8 = nc.declare_dram_parameter("at8", [P, KT, SC], F8, isOutput=False)
    av8 = nc.declare_dram_parameter("av8", [P, KTX, AVW], F8, isOutput=False)
    out16 = nc.declare_dram_parameter("out16", [P, NT, DV], F16, isOutput=True)

    # exp/X only need columns for real (unmasked) output rows; the rest of
    # each X row block is zeroed once and the garbage output rows discarded.
    slabs = [(lo, min(512, EW - lo)) for lo in range(0, EW, 512)]

    with tile.TileContext(nc) as tc:
        with (
            tc.tile_pool(name="w", bufs=1) as wpool,
            tc.tile_pool(name="x", bufs=1) as xpool,
            tc.tile_pool(name="e", bufs=2 * NT) as epool,
            tc.tile_pool(name="qkps", bufs=4, space="PSUM") as qkps,
            tc.tile_pool(name="avps", bufs=1, space="PSUM") as avps,
            tc.tile_pool(name="o", bufs=1) as opool,
        ):
            # warm-up operands for the PE p-state ramp (zeroed)
            warm_l = wpool.tile([P, 2, 64], F8, name="warm_l", tag="warm_l")
            nc.gpsimd.memset(warm_l[:, :, :], 0.0)
            warm_r = wpool.tile([P, 2, 512], F8, name="warm_r", tag="warm_r")
            nc.gpsimd.memset(warm_r[:, :, :], 0.0)

            # at8 split across two hardware DMA queues (SP + Activation)
            at8t = wpool.tile([P, KT, SC], F8, name="at8t", tag="at8t")
            KC = 4  # k-tiles in the first chunk (covers DoubleRow pairs 0,1)
            nc.sync.dma_start(at8t[:, 0:KC, :], at8[:, 0:KC, :])
            nc.scalar.dma_start(at8t[:, KC:KT, :], at8[:, KC:KT, :])
            av8t = wpool.tile([P, KTX, AVW], F8, name="av8t", tag="av8t")
            nc.scalar.dma_start(av8t[:, :, :], av8[:, :, :])

            # X tiles; k-slot NT..KTX-1 stays zero (contraction padding)
            xall = xpool.tile([P, KTX, SC], F8, name="xall", tag="xall")
            for t in range(NT, KTX):
                nc.gpsimd.memset(xall[:, t, :], 0.0)
            if EW < SC:
                for t in range(NT):
                    nc.gpsimd.memset(xall[:, t, EW:SC], 0.0)

            # --- PE warm-up: dependency-free dummy matmuls keep the tensor
            #     engine busy (ramping its p-state) while the inputs stream ---
            wps = qkps.tile([P, 512], F32, tag="qk", name="qk_warm")
            for _ in range(6):
                nc.tensor.matmul(
                    wps[0:64, :], warm_l[:, :, :], warm_r[:, :, :], perf_mode=DR
                )

            # --- QK phase (slab-major): psum = (8A)(8A)^T block,
            #     X = 16*(exp(psum/64) - 1).
            # The pool-engine -1/scale pass runs in 128-column chunks,
            # column-major, so each AV m-block's weight load waits only on
            # the two chunk conversions it actually reads — the AV phase
            # starts as soon as the last row block's first chunk lands. ---
            ets = {}
            for lo, w in slabs:
                for i in range(NT):
                    ps = qkps.tile([P, 512], F32, tag="qk", name=f"qk_{i}_{lo}")
                    for q in range(KT // 2):
                        nc.tensor.matmul(
                            ps[:, :w],
                            at8t[:, 2 * q : 2 * q + 2, i * P : (i + 1) * P],
                            at8t[:, 2 * q : 2 * q + 2, lo : lo + w],
                            start=(q == 0),
                            stop=(q == KT // 2 - 1),
                            perf_mode=DR,
                        )
                    et = epool.tile([P, 512], F16, tag="et", name=f"et_{i}_{lo}")
                    nc.scalar.activation(
                        et[:, :w], ps[:, :w], EXPF, scale=1.0 / (ASC * ASC)
                    )
                    ets[(lo, i)] = et
                    nc.gpsimd.tensor_scalar(
                        xall[:, i, lo : lo + w],
                        et[:, :w],
                        -1.0,
                        XSC,
                        mybir.AluOpType.add,
                        mybir.AluOpType.mult,
                    )

            # --- AV phase: P_ext[m] = X^T-block @ [8V | 8*ones] ---
            ots = [
                opool.tile([P, DV], F16, name=f"og{m}", tag=f"og{m}")
                for m in range(NT)
            ]
            pabuf = [
                avps.tile([P, 512], F32, tag=f"pa{x}", name=f"pa{x}") for x in range(2)
            ]
            pbbuf = [
                avps.tile([P, 258], F32, tag=f"pb{x}", name=f"pb{x}") for x in range(2)
            ]
            NQ = KTX // 2  # DoubleRow contraction pairs (incl. zero padding)
            for m in range(NT):
                pa = pabuf[m % 2]
                pb = pbbuf[m % 2]
                for q in range(NQ):
                    lt = xall[:, 2 * q : 2 * q + 2, m * P : (m + 1) * P]
                    nc.tensor.matmul(
                        pa[:, :],
                        lt,
                        av8t[:, 2 * q : 2 * q + 2, 0:512],
                        start=(q == 0),
                        stop=(q == NQ - 1),
                        perf_mode=DR,
                    )
                    nc.tensor.matmul(
                        pb[:, :],
                        lt,
                        av8t[:, 2 * q : 2 * q + 2, 512:770],
                        start=(q == 0),
                        stop=(q == NQ - 1),
                        perf_mode=DR,
                    )
                nc.vector.tensor_scalar_add(ots[m][:, 0:512], pa[:, :], 0.0)
                nc.vector.tensor_scalar_add(ots[m][:, 512:DV], pb[:, 0:257], 0.0)
                nc.sync.dma_start(out16[:, m, :], ots[m][:, :])

    return nc


def _get_nc(SC, EW):
    if (SC, EW) not in _cache:
        _cache[(SC, EW)] = _build(SC, EW)
    return _cache[(SC, EW)]


def kernel(input_a, input_mask, _trace=False, **_kw):
    A = np.asarray(input_a, dtype=np.float32)  # [B, S, D]
    M = np.asarray(input_mask)  # [B, S] int32

    nb, s, d = A.shape
    idxs = [np.nonzero(M[b] != 0)[0] for b in range(nb)]
    maxn = max(1, max(len(ix) for ix in idxs))
    SC = max(P, -(-maxn // P) * P)  # pad count to multiple of 128
    EW = min(SC, -(-maxn // 32) * 32)  # exp width: real rows padded to 32
    NT = SC // P
    KT_ = d // P
    KTX = 2 * ((NT + 1) // 2)

    in_maps = []
    hosts = []
    for b in range(nb):
        ix = idxs[b]
        n = len(ix)
        Ac = np.zeros((SC, d), np.float32)
        Ac[:n] = A[b][ix]
        a8 = (ASC * Ac).astype(NP8)  # [SC, d] fp8 of 8*A
        a8f = a8.astype(np.float32)
        at8 = np.ascontiguousarray(
            a8.T.reshape(KT_, P, SC).transpose(1, 0, 2)
        )
        av8 = np.zeros((P, KTX, AVW), NP8)
        blk = np.zeros((SC, AVW), NP8)
        blk[:, :d] = a8
        blk[:n, d] = ASC
        av8[:, 0:NT, :] = blk.reshape(NT, P, AVW).transpose(1, 0, 2)
        in_maps.append({"at8": at8, "av8": av8})

        # host-side correction data
        colsum = Ac[:n].sum(axis=0)  # true fp32 column sums
        qk_ss = np.einsum("ij,ij->i", a8f[:n], a8f[:n])
        x_dev = (
            (
                (
                    np.exp(qk_ss / (ASC * ASC)).astype(np.float16).astype(np.float32)
                    - 1.0
                )
                * XSC
            ).astype(NP8)
        ).astype(np.float32) / XSC
        a64 = Ac[:n].astype(np.float64)
        x_ideal = np.expm1(np.einsum("ij,ij->i", a64, a64)).astype(np.float32)
        hosts.append((colsum, x_dev, x_ideal, a8f))

    nc = _get_nc(SC, EW)
    res = bass_utils.run_bass_kernel_spmd(
        nc, in_maps, core_ids=list(range(NCORES)), trace=_trace
    )

    scale = 1.0 / (ASC * XSC)
    out = np.zeros((nb, s, d), np.float32)
    for b in range(nb):
        ix = idxs[b]
        n = len(ix)
        colsum, x_dev, x_ideal, a8f = hosts[b]
        R = (
            res.results[b]["out16"]
            .transpose(1, 0, 2)
            .reshape(SC, DV)
            .astype(np.float32)
        )
        Vq = a8f[:n] / ASC
        Ac_n = A[b][ix]
        U = (
            R[:n, :d] * scale
            + colsum[None, :]
            - x_dev[:, None] * Vq
            + x_ideal[:, None] * Ac_n
        )
        rs = R[:n, d] * scale + n + (x_ideal - x_dev)
        out[b][ix] = U / (rs + EPS)[:, None]
    if _trace:
        kernel.last_results = res
    return out
